# revision 1
# baseline (speedup 1.0000x reference)
"""GAT layer kernel for Trainium2, SPMD across 8 NeuronCores.

Math (per batch b):
    h[n]   = x[b,n] @ proj_w[n] + proj_b[n]
    s[i,j] = h[i] . a_src[j] + h[j] . a_dst[j]
    att    = softmax_j( mask(leaky_relu(s)) ),  mask: (0<dist<0.5)|eye
    y[i]   = sum_j att[i,j] h[j]

Sharding: destination rows i are split into 8 blocks of 512. Each core
projects its own 512 nodes (1/8 of the 64MB proj_w read), the projected
features are all-gathered (540KB/rank), and each core computes its row
block of scores/softmax/aggregation, flash-style (the [B,N,N] score
tensor never leaves PSUM/SBUF).

The kernel is core-agnostic: the host pre-slices every input and patches
the adjacency diagonal directly into the dist rows, and the AllGather
orders shards by rank.
"""

import numpy as np

B = 4
N = 4096
C = 64
R = 8            # cores
NB = N // R      # 512 rows per core
JT = 128         # j-tile width (partition dim of transposed scores)
NJT = N // JT    # 32 j-tiles
GJ = 2           # j-tiles per PSUM/elementwise group
NG = NJT // GJ   # 16 groups
C1 = C + 1       # h + ones column
ALPHA = 0.01
NEG = -1.0e15

Z1 = B * NB * C1     # region 1 of gather payload: [b][n][h(64), 1.0]
Z2 = B * NB          # region 2: d[b][n]
ZT = Z1 + Z2

_CACHE = {}


def _build():
    import concourse.bass as bass
    import concourse.tile as tile
    from concourse import bacc, mybir
    from concourse.masks import make_identity

    f32 = mybir.dt.float32
    Alu = mybir.AluOpType
    Act = mybir.ActivationFunctionType

    nc = bacc.Bacc("TRN2", target_bir_lowering=False, debug=False, num_devices=R)

    xb = nc.dram_tensor("xb", [B, NB, C], f32, kind="ExternalInput").ap()
    db = nc.dram_tensor("db", [NB, N], f32, kind="ExternalInput").ap()
    wb = nc.dram_tensor("wb", [NB, C, C], f32, kind="ExternalInput").ap()
    bb = nc.dram_tensor("bb", [NB, C], f32, kind="ExternalInput").ap()
    aw = nc.dram_tensor("aw", [N, 2 * C], f32, kind="ExternalInput").ap()
    adb = nc.dram_tensor("adb", [NB, C], f32, kind="ExternalInput").ap()
    out = nc.dram_tensor("out", [B, NB, C], f32, kind="ExternalOutput").ap()

    z_local = nc.dram_tensor("z_local", [ZT], f32, kind="Internal")
    z_full = nc.dram_tensor("z_full", [R, ZT], f32, kind="Internal",
                            addr_space="Shared")

    with tile.TileContext(nc) as tc:
        _body(tc, nc, bass, mybir, make_identity, f32, Alu, Act,
              xb, db, wb, bb, aw, adb, out, z_local, z_full)
    nc.compile()  # bacc lowering: register allocation, DCE, nop-fusion
    return nc


def _body(tc, nc, bass, mybir, make_identity, f32, Alu, Act,
          xb, db, wb, bb, aw, adb, out, z_local, z_full):
    from contextlib import ExitStack
    import os
    n_groups = int(os.environ.get("GAT_NGROUPS", "4"))
    n_batch = int(os.environ.get("GAT_NBATCH", str(B)))
    skip_mask = os.environ.get("GAT_SKIP_MASK", "0") == "1"
    skip_epi = os.environ.get("GAT_SKIP_EPI", "0") == "1"
    skip_main = os.environ.get("GAT_SKIP_MAIN", "0") == "1"
    reps = int(os.environ.get("GAT_REPS", "1"))
    skip_ew = os.environ.get("GAT_SKIP_EW", "0") == "1"
    skip_agg = os.environ.get("GAT_SKIP_AGG", "0") == "1"
    skip_scores = os.environ.get("GAT_SKIP_SCORES", "0") == "1"
    hoist_dma = os.environ.get("GAT_HOIST_DMA", "0") == "1"

    ctx = ExitStack()
    with ctx:
        const = ctx.enter_context(tc.tile_pool(name="const", bufs=1))
        pa_w = ctx.enter_context(tc.tile_pool(name="pa_w", bufs=4))
        pa_x = ctx.enter_context(tc.tile_pool(name="pa_x", bufs=4))
        pa_sb = ctx.enter_context(tc.tile_pool(name="pa_sb", bufs=2))
        mk_sb = ctx.enter_context(tc.tile_pool(name="mk_sb", bufs=3))
        mn_sb = ctx.enter_context(tc.tile_pool(name="mn_sb", bufs=2))
        ew_sb = ctx.enter_context(tc.tile_pool(name="ew_sb", bufs=3))
        ep_sb = ctx.enter_context(tc.tile_pool(name="ep_sb", bufs=2))
        ps_s = ctx.enter_context(tc.tile_pool(name="ps_s", bufs=2, space="PSUM"))
        ps_y = ctx.enter_context(tc.tile_pool(name="ps_y", bufs=2, space="PSUM"))
        ps_t = ctx.enter_context(tc.tile_pool(name="ps_t", bufs=2, space="PSUM"))

        ident = const.tile([128, 128], f32)
        make_identity(nc, ident[:])

        # big resident mask: m_all[:, jt*NB:(jt+1)*NB] is M^T for j-tile jt
        m_all = const.tile([128, NJT * NB], f32)

        # ---------------- Phase A: project local nodes ----------------
        # wb viewed as [(n c), o]; xb as [(n c), b]
        wb_f = wb.rearrange("n c o -> (n c) o")
        xb_f = xb.rearrange("b n c -> (n c) b")
        z1w = z_local.ap()[0:Z1].rearrange("(b n c) -> n b c", b=B, n=NB)
        z2w = z_local.ap()[Z1:ZT].rearrange("(b n) -> n b", b=B)

        for g in range(n_groups):               # groups of 128 local nodes
            n0 = g * 128
            psum_h = ps_y.tile([64, 128, B], f32, tag="psy")
            for t in range(64):                 # node pairs (2t, 2t+1)
                nn = n0 + 2 * t
                w_pair = pa_w.tile([128, C], f32, tag="wp")
                nc.sync.dma_start(out=w_pair[:], in_=wb_f[nn * C:(nn + 2) * C, :])
                x_pair = pa_x.tile([128, B], f32, tag="xp")
                nc.sync.dma_start(out=x_pair[:], in_=xb_f[nn * C:(nn + 2) * C, :])
                nc.tensor.matmul(psum_h[:, 2 * t, :],
                                 w_pair[0:64, :], x_pair[0:64, :],
                                 start=True, stop=True)
                nc.tensor.matmul(psum_h[:, 2 * t + 1, :],
                                 w_pair[64:128, :], x_pair[64:128, :],
                                 start=True, stop=True)
            # psum_h[o, n, b] = h[b, n0+n, o] (pre-bias)
            hT_sb = pa_sb.tile([64, 128, B], f32, tag="hts")
            nc.vector.tensor_copy(hT_sb[:], psum_h[:])

            bias_g = pa_sb.tile([128, C], f32, tag="biasg")
            nc.sync.dma_start(out=bias_g[:], in_=bb[n0:n0 + 128, :])
            adst_g = pa_sb.tile([128, C], f32, tag="adstg")
            nc.sync.dma_start(out=adst_g[:], in_=adb[n0:n0 + 128, :])

            h_nat = pa_sb.tile([128, B, C1], f32, tag="hnat")
            d_g = pa_sb.tile([128, B], f32, tag="dg")
            dtmp = pa_sb.tile([128, C], f32, tag="dtmp")
            for b in range(B):
                psum_t2 = ps_t.tile([128, 64], f32, tag="pst")
                nc.tensor.transpose(psum_t2[:],
                                    hT_sb[:, :, b],
                                    ident[0:64, 0:64])
                nc.vector.tensor_add(h_nat[:, b, 0:C], psum_t2[:], bias_g[:])
                nc.vector.memset(h_nat[:, b, C:C1], 1.0)
                nc.vector.tensor_mul(dtmp[:], h_nat[:, b, 0:C], adst_g[:])
                nc.vector.reduce_sum(d_g[:, b:b + 1], dtmp[:],
                                     axis=mybir.AxisListType.X)
            nc.sync.dma_start(out=z1w[n0:n0 + 128, :, :], in_=h_nat[:])
            nc.sync.dma_start(out=z2w[n0:n0 + 128, :], in_=d_g[:])

        # ---------------- Phase C: adjacency mask (independent) ----------------
        # M^T[j, i] = 0 if (0 < dist[i,j] < 0.5) else -1e15 (diag pre-patched on host)
        dbT = db.rearrange("i j -> j i")
        if skip_mask:
            nc.vector.memset(m_all[:], 0.0)
        for jt in range(0 if not skip_mask else NJT, NJT):
            j0 = jt * JT
            dt = mk_sb.tile([128, NB], f32, tag="dt")
            nc.sync.dma_start(out=dt[:], in_=dbT[j0:j0 + JT, :])
            t1 = mk_sb.tile([128, NB], f32, tag="t1")
            nc.vector.tensor_scalar(t1[:], dt[:], 0.5, None, Alu.is_lt)
            em = mk_sb.tile([128, NB], f32, tag="em")
            nc.vector.scalar_tensor_tensor(
                out=em[:], in0=dt[:], scalar=0.0, in1=t1[:],
                op0=Alu.is_gt, op1=Alu.mult)
            nc.vector.tensor_scalar(m_all[:, jt * NB:(jt + 1) * NB],
                                    em[:], 1.0, -NEG, Alu.subtract, Alu.mult)

        # ---------------- Phase B: all-gather projected features ----------------
        nc.gpsimd.collective_compute(
            "AllGather",
            Alu.bypass,
            replica_groups=[list(range(R))],
            ins=[z_local.ap().opt()],
            outs=[z_full.ap().opt()],
        )

        # views of the gathered payload
        awT = aw.rearrange("j c -> c j")
        z1l = z_local.ap()[0:Z1].rearrange("(b n c) -> c b n", b=B, n=NB)

        # ---------------- Phase D: scores / softmax / aggregation ----------------
        hoisted = {}
        for b in [bb2 for _rep in range(reps)
                  for bb2 in range(n_batch if not skip_main else 0)]:
            if hoist_dma and hoisted:
                waug, rhs_hh, v_all = hoisted["w"], hoisted["r"], hoisted["v"]
            else:
                # waug[c', j]: rows 0..63 = a_src^T, row 64 = d[b, :]
                waug = mn_sb.tile([C1, N], f32, tag="waug")
                nc.sync.dma_start(out=waug[0:C, :], in_=awT[0:C, :])
                for r in range(R):
                    zf2 = z_full.ap()[r, Z1:ZT].rearrange("(b n) -> b n", b=B)
                    nc.sync.dma_start(out=waug[C:C1, r * NB:(r + 1) * NB],
                                      in_=zf2[b:b + 1, :])
                # rhs_hh[c', i]: own block's h^T plus ones row (from z_local)
                rhs_hh = mn_sb.tile([C1, NB], f32, tag="rhs")
                nc.sync.dma_start(out=rhs_hh[:], in_=z1l[:, b, :])
                # V' tiles for all j: [128, 65] per j-tile (h + ones)
                v_all = mn_sb.tile([128, NJT * C1], f32, tag="vall")
                for jt in range(NJT):
                    r, nn = divmod(jt * JT, NB)
                    zf1 = z_full.ap()[r, 0:Z1].rearrange("(b n c) -> n b c", b=B, n=NB)
                    nc.sync.dma_start(out=v_all[:, jt * C1:(jt + 1) * C1],
                                      in_=zf1[nn:nn + JT, b, :])
                hoisted = {"w": waug, "r": rhs_hh, "v": v_all}

            psum_y = ps_y.tile([C1, NB], f32, tag="psy")
            for g in range(NG):
                psum_s = ps_s.tile([128, GJ * NB], f32, tag="pss")
                if not skip_scores:
                    for q in range(GJ):
                        jt = g * GJ + q
                        nc.tensor.matmul(psum_s[:, q * NB:(q + 1) * NB],
                                         waug[:, jt * JT:(jt + 1) * JT],
                                         rhs_hh[:],
                                         start=True, stop=True)
                # v = s + M  (masked scores), u = leaky(v), p = exp(u)
                if skip_ew:
                    p = ew_sb.tile([128, GJ * NB], f32, tag="pt")
                    nc.vector.tensor_add(p[:], psum_s[:],
                                         m_all[:, g * GJ * NB:(g + 1) * GJ * NB])
                else:
                    v = ew_sb.tile([128, GJ * NB], f32, tag="vt")
                    nc.vector.tensor_add(v[:], psum_s[:],
                                         m_all[:, g * GJ * NB:(g + 1) * GJ * NB])
                    u = ew_sb.tile([128, GJ * NB], f32, tag="ut")
                    nc.vector.scalar_tensor_tensor(
                        out=u[:], in0=v[:], scalar=ALPHA, in1=v[:],
                        op0=Alu.mult, op1=Alu.max)
                    p = ew_sb.tile([128, GJ * NB], f32, tag="pt")
                    nc.scalar.activation(p[:], u[:], Act.Exp)
                if not skip_agg:
                    for q in range(GJ):
                        jt = g * GJ + q
                        nc.tensor.matmul(psum_y[:],
                                         v_all[:, jt * C1:(jt + 1) * C1],
                                         p[:, q * NB:(q + 1) * NB],
                                         start=(jt == 0), stop=(jt == NJT - 1))
                elif g == 0:
                    nc.tensor.matmul(psum_y[:], v_all[:, 0:C1], p[:, 0:NB],
                                     start=True, stop=True)

            # ---------------- normalize + write out ----------------
            if skip_epi:
                continue
            y_sb = ep_sb.tile([C1, NB], f32, tag="ysb")
            nc.vector.tensor_copy(y_sb[:], psum_y[:])
            for g4 in range(4):
                psum_t = ps_t.tile([128, C1], f32, tag="pst")
                nc.tensor.transpose(psum_t[:],
                                    y_sb[:, g4 * 128:(g4 + 1) * 128],
                                    ident[0:C1, 0:C1])
                rec = ep_sb.tile([128, 1], f32, tag="rec")
                nc.vector.reciprocal(rec[:], psum_t[:, C:C1])
                y_out = ep_sb.tile([128, C], f32, tag="yout")
                nc.vector.tensor_scalar(y_out[:], psum_t[:, 0:C], rec[:],
                                        None, Alu.mult)
                nc.sync.dma_start(out=out[b, g4 * 128:(g4 + 1) * 128, :],
                                  in_=y_out[:])


def _get_nc():
    if "nc" not in _CACHE:
        _CACHE["nc"] = _build()
    return _CACHE["nc"]


def _make_in_maps(inputs):
    x = np.asarray(inputs["x"], dtype=np.float32)
    dist_mat = np.asarray(inputs["dist_mat"], dtype=np.float32)
    proj_w = np.asarray(inputs["proj_w"], dtype=np.float32)
    proj_b = np.asarray(inputs["proj_b"], dtype=np.float32)
    a_w = np.ascontiguousarray(np.asarray(inputs["a_w"], dtype=np.float32))

    in_maps = []
    idx = np.arange(NB)
    for k in range(R):
        blk = slice(k * NB, (k + 1) * NB)
        dbk = np.array(dist_mat[blk, :], dtype=np.float32, copy=True)
        dbk[idx, k * NB + idx] = 0.25  # force diagonal -> edge (adj |= eye)
        in_maps.append({
            "xb": np.ascontiguousarray(x[:, blk, :]),
            "db": dbk,
            "wb": np.ascontiguousarray(proj_w[blk]),
            "bb": np.ascontiguousarray(proj_b[blk]),
            "aw": a_w,
            "adb": np.ascontiguousarray(a_w[blk, C:]),
        })
    return in_maps


def kernel(x, dist_mat, proj_w, proj_b, a_w):
    from concourse.bass_utils import run_bass_kernel_spmd

    nc = _get_nc()
    in_maps = _make_in_maps({"x": x, "dist_mat": dist_mat, "proj_w": proj_w,
                             "proj_b": proj_b, "a_w": a_w})
    last_err = None
    for _attempt in range(3):
        try:
            res = run_bass_kernel_spmd(nc, in_maps, core_ids=list(range(R)))
            outs = [res.results[k]["out"] for k in range(R)]
            return np.concatenate(outs, axis=1).astype(np.float32)
        except Exception as e:  # transient runtime/device errors: retry
            last_err = e
    raise last_err



# revision 18
# speedup vs baseline: 1.0312x; 1.0312x over previous
"""GAT layer kernel for Trainium2, SPMD across 8 NeuronCores.

Math (per batch b):
    h[n]   = x[b,n] @ proj_w[n] + proj_b[n]
    s[i,j] = h[i] . a_src[j] + h[j] . a_dst[j]
    att    = softmax_j( mask(leaky_relu(s)) ),  mask: (0<dist<0.5)|eye
    y[i]   = sum_j att[i,j] h[j]

Sharding: destination rows i are split into 8 blocks of 512 per core.
Each core projects its own 512 nodes, the projected features are
all-gathered (bf16), and each core computes its row block of
scores/softmax/aggregation flash-style (scores stay in PSUM/SBUF).

All DMA access patterns are contiguous: the host pre-transposes
dist (to [j, i] per block), a_src (to [c, j]), x (to [c2, pair, b])
and proj_b (to [c, n]) so the device never does element-granular
strided gathers. The adjacency mask enters the scores as a TensorE
accumulation (-1e15*I @ notE^T) so no DVE pass over the score matrix
is needed; LeakyReLU is split between ScalarE (Lrelu) and VectorE to
balance engine time; exp runs on ScalarE in bf16.
"""

import numpy as np
import ml_dtypes

BF16 = ml_dtypes.bfloat16

B = 4
N = 4096
C = 64
R = 8            # cores
NB = N // R      # 512 rows per core
JT = 128         # j-tile width (partition dim of transposed scores)
NJT = N // JT    # 32 j-tiles
GJ = 2           # j-tiles per PSUM/elementwise group
NG = NJT // GJ   # 16 groups
C1 = C + 1       # h + ones column
NP = NB // 2     # node pairs per core (256)
ALPHA = 0.01
NEG = -1.0e15

Z1 = NB * B * C1     # region 1 of gather payload: [n][b][h(64), 1.0]
Z2 = B * NB          # region 2: d[b][n]
ZT = Z1 + Z2

_CACHE = {}


def _build():
    import concourse.bass as bass
    import concourse.tile as tile
    from concourse import bacc, mybir
    from concourse.masks import make_identity

    f32 = mybir.dt.float32
    bf16 = mybir.dt.bfloat16
    Alu = mybir.AluOpType
    Act = mybir.ActivationFunctionType

    nc = bacc.Bacc("TRN2", target_bir_lowering=False, debug=False, num_devices=R)

    wb = nc.dram_tensor("wb", [NB * C, C], f32, kind="ExternalInput").ap()
    xq = nc.dram_tensor("xq", [NP, 2 * C, B], f32, kind="ExternalInput").ap()
    bbT = nc.dram_tensor("bbT", [C, NB], f32, kind="ExternalInput").ap()
    adT = nc.dram_tensor("adT", [NB, C], bf16, kind="ExternalInput").ap()
    dbT = nc.dram_tensor("dbT", [N, NB], f32, kind="ExternalInput").ap()
    asT = nc.dram_tensor("asT", [C, N], bf16, kind="ExternalInput").ap()
    out = nc.dram_tensor("out", [B, NB, C], f32, kind="ExternalOutput").ap()

    z_local = nc.dram_tensor("z_local", [ZT], bf16, kind="Internal")
    z_full = nc.dram_tensor("z_full", [R, ZT], bf16, kind="Internal",
                            addr_space="Shared")

    with tile.TileContext(nc) as tc:
        _body(tc, nc, bass, mybir, make_identity, f32, bf16, Alu, Act,
              wb, xq, bbT, adT, dbT, asT, out, z_local, z_full)
    nc.compile()
    return nc


def _body(tc, nc, bass, mybir, make_identity, f32, bf16, Alu, Act,
          wb, xq, bbT, adT, dbT, asT, out, z_local, z_full):
    from contextlib import ExitStack
    import os
    n_groups = int(os.environ.get("GAT_NGROUPS", "4"))
    n_batch = int(os.environ.get("GAT_NBATCH", str(B)))
    skip_main = os.environ.get("GAT_SKIP_MAIN", "0") == "1"
    skip_coll = os.environ.get("GAT_SKIP_COLL", "0") == "1"
    skip_vall = os.environ.get("GAT_SKIP_VALL", "0") == "1"
    skip_mask = os.environ.get("GAT_SKIP_MASK", "0") == "1"
    skip_phase_a = os.environ.get("GAT_SKIP_PHASE_A", "0") == "1"
    pa_level = int(os.environ.get("GAT_PA_LEVEL", "7"))

    ctx = ExitStack()
    with ctx:
        const = ctx.enter_context(tc.tile_pool(name="const", bufs=1))
        pa_w = ctx.enter_context(tc.tile_pool(name="pa_w", bufs=4))
        pa_x = ctx.enter_context(tc.tile_pool(name="pa_x", bufs=4))
        pa_sb = ctx.enter_context(tc.tile_pool(name="pa_sb", bufs=2))
        mk_sb = ctx.enter_context(tc.tile_pool(name="mk_sb", bufs=3))
        mn_sb = ctx.enter_context(tc.tile_pool(name="mn_sb", bufs=2))
        ew_sb = ctx.enter_context(tc.tile_pool(name="ew_sb", bufs=3))
        ep_sb = ctx.enter_context(tc.tile_pool(name="ep_sb", bufs=2))
        ps_s = ctx.enter_context(tc.tile_pool(name="ps_s", bufs=2, space="PSUM"))
        ps_y = ctx.enter_context(tc.tile_pool(name="ps_y", bufs=2, space="PSUM"))
        ps_t = ctx.enter_context(tc.tile_pool(name="ps_t", bufs=2, space="PSUM"))

        min_mode = int(os.environ.get("GAT_MIN", "0"))
        identf = const.tile([128, 128], f32)
        identb = const.tile([128, 128], bf16)
        iNeg = const.tile([128, 128], bf16)
        if min_mode < 1:
            make_identity(nc, identf[:])
            nc.vector.tensor_copy(identb[:], identf[:])
            # iNeg = -1e15 * I: mask enters scores as iNeg.T @ notE^T
            nc.vector.tensor_scalar(iNeg[:], identb[:], -NEG, -1.0,
                                    Alu.mult, Alu.mult)

        # resident tensors
        bbT_res = const.tile([C, NB], f32)
        adT_res = const.tile([128, 4, C], bf16)
        if os.environ.get("GAT_SKIP_ADT", "0") != "1" and min_mode < 2:
            for g in range(4):
                nc.sync.dma_start(out=adT_res[:, g, :],
                                  in_=adT[g * 128:(g + 1) * 128, :])
        if os.environ.get("GAT_SKIP_BBT", "0") != "1" and min_mode < 2:
            bb_split = int(os.environ.get("GAT_BBT_SPLIT", "2"))
            step = NB // bb_split
            for s in range(bb_split):
                nc.sync.dma_start(out=bbT_res[:, s * step:(s + 1) * step],
                                  in_=bbT[:, s * step:(s + 1) * step])

        # h^T (post-bias) + ones row, resident: rhs of the score matmuls
        hT_res = const.tile([C1, B, NB], bf16)
        if min_mode < 3:
            nc.vector.memset(hT_res[C:C1, :, :], 1.0)
        # notE^T resident: 1.0 where NOT an edge (j on partitions)
        ne_all = const.tile([128, NJT * NB], bf16)
        # gathered V' tiles for all (j-tile, b): [128, 65] slices
        v_all = const.tile([128, NJT, B * C1], bf16)
        # per-node attention-dst dot, all groups: d_all[:, b*4+g]
        d_all = const.tile([128, 16], f32)

        z1w = z_local.ap()[0:Z1].rearrange("(n b c) -> n b c", n=NB, b=B)

        # ---------------- Phase A: project local nodes ----------------
        for g in range(0 if skip_phase_a else n_groups):               # groups of 128 local nodes
            n0 = g * 128
            psum_h = ps_y.tile([C, 128, B], f32, tag="psy")
            for t in range(64):                 # node pairs (2t, 2t+1)
                tg = g * 64 + t
                w_pair = pa_w.tile([128, C], f32, tag="wp")
                nc.sync.dma_start(out=w_pair[:],
                                  in_=wb[(2 * tg) * C:(2 * tg + 2) * C, :])
                x_pair = pa_x.tile([128, B], f32, tag="xp")
                nc.sync.dma_start(out=x_pair[:], in_=xq[tg, :, :])
                if pa_level < 2:
                    continue
                nc.tensor.matmul(psum_h[:, 2 * t, :],
                                 w_pair[0:C, :], x_pair[0:C, :],
                                 start=True, stop=True)
                nc.tensor.matmul(psum_h[:, 2 * t + 1, :],
                                 w_pair[C:2 * C, :], x_pair[C:2 * C, :],
                                 start=True, stop=True)
            # psum_h[o, n, b] = h[b, n0+n, o] (pre-bias)
            if pa_level < 3:
                continue
            for b in range(B):
                nc.vector.tensor_add(hT_res[0:C, b, n0:n0 + 128],
                                     psum_h[:, :, b],
                                     bbT_res[:, n0:n0 + 128])

            h_nat = pa_sb.tile([128, B, C1], bf16, tag="hnat")
            nc.vector.memset(h_nat[:, :, C:C1], 1.0)
            dtmp = pa_sb.tile([128, C], f32, tag="dtmp")
            for b in range(B):
                if pa_level < 4:
                    continue
                pst = ps_t.tile([128, C], bf16, tag="pst")
                nc.tensor.transpose(pst[:],
                                    hT_res[0:C, b, n0:n0 + 128],
                                    identb[0:C, 0:C])
                nc.vector.tensor_copy(h_nat[:, b, 0:C], pst[:])
                if pa_level < 5:
                    continue
                nc.vector.tensor_mul(dtmp[:], h_nat[:, b, 0:C],
                                     adT_res[:, g, :])
                nc.vector.reduce_sum(d_all[:, b * 4 + g:b * 4 + g + 1],
                                     dtmp[:], axis=mybir.AxisListType.X)
            if pa_level < 6:
                continue
            if os.environ.get("GAT_Z_GPSIMD", "1") == "1":
                nc.gpsimd.dma_start(out=z1w[n0:n0 + 128, :, :], in_=h_nat[:])
            else:
                nc.sync.dma_start(out=z1w[n0:n0 + 128, :, :], in_=h_nat[:])

        # d values -> z region 2 as [b][n] rows (transpose on PE)
        if skip_phase_a:
            nc.vector.memset(d_all[:], 0.0)
        if pa_level >= 7:
            pst2 = ps_t.tile([16, 128], f32, tag="pst")
            nc.tensor.transpose(pst2[:], d_all[:], identf[:])
            dT_sb = pa_sb.tile([16, 128], bf16, tag="dts")
            nc.vector.tensor_copy(dT_sb[:], pst2[:])
            for b in range(B):
                z2v = z_local.ap()[Z1 + b * NB:Z1 + (b + 1) * NB].rearrange(
                    "(g n) -> g n", g=4)
                nc.gpsimd.dma_start(out=z2v, in_=dT_sb[b * 4:(b + 1) * 4, :])

        # ---------------- Phase C: adjacency mask (independent) ----------------
        # notE^T[j, i] = 1.0 if NOT ((0 < dist[i,j] < 0.5)) else 0.0
        # (diagonal pre-patched to 0.25 on host => edge)
        if skip_mask:
            nc.vector.memset(ne_all[:], 0.0)
        for jt in range(0 if skip_mask else NJT):
            j0 = jt * JT
            dt_ = mk_sb.tile([128, NB], f32, tag="dt")
            nc.sync.dma_start(out=dt_[:], in_=dbT[j0:j0 + JT, :])
            t1 = mk_sb.tile([128, NB], f32, tag="t1")
            nc.vector.tensor_scalar(t1[:], dt_[:], 0.5, None, Alu.is_ge)
            nc.vector.scalar_tensor_tensor(
                out=ne_all[:, jt * NB:(jt + 1) * NB],
                in0=dt_[:], scalar=0.0, in1=t1[:],
                op0=Alu.is_le, op1=Alu.max)

        # ---------------- Phase B: all-gather projected features ----------------
        if not skip_coll:
            nc.gpsimd.collective_compute(
                "AllGather",
                mybir.AluOpType.bypass,
                replica_groups=[list(range(R))],
                ins=[z_local.ap().opt()],
                outs=[z_full.ap().opt()],
            )

        # V' tiles for all j-tiles/batches (one contiguous DMA per j-tile)
        if skip_vall:
            nc.vector.memset(v_all[:], 0.0)
        for r in range(0 if skip_vall else R):
            z1f = z_full.ap()[r, 0:Z1].rearrange("(n b c) -> n (b c)", n=NB, b=B)
            for jl in range(4):
                nc.sync.dma_start(out=v_all[:, r * 4 + jl, :],
                                  in_=z1f[jl * 128:(jl + 1) * 128, :])

        # ---------------- Phase D: scores / softmax / aggregation ----------------
        for b in range(n_batch if not skip_main else 0):
            # waug[c', j]: rows 0..63 = a_src^T, row 64 = d[b, :]
            waug = mn_sb.tile([C1, N], bf16, tag="waug")
            for s in range(8):
                nc.sync.dma_start(out=waug[0:C, s * NB:(s + 1) * NB],
                                  in_=asT[:, s * NB:(s + 1) * NB])
            for r in range(0 if skip_coll else R):
                zd = z_full.ap()[r, Z1 + b * NB:Z1 + (b + 1) * NB].rearrange(
                    "(o n) -> o n", o=1)
                nc.sync.dma_start(out=waug[C:C1, r * NB:(r + 1) * NB], in_=zd)

            psum_y = ps_y.tile([C1, NB], f32, tag="psy")
            for g in range(NG):
                psum_s = ps_s.tile([128, GJ * NB], f32, tag="pss")
                for q in range(GJ):
                    jt = g * GJ + q
                    nc.tensor.matmul(psum_s[:, q * NB:(q + 1) * NB],
                                     waug[:, jt * JT:(jt + 1) * JT],
                                     hT_res[:, b, :],
                                     start=True, stop=False)
                    nc.tensor.matmul(psum_s[:, q * NB:(q + 1) * NB],
                                     iNeg[:],
                                     ne_all[:, jt * NB:(jt + 1) * NB],
                                     start=False, stop=True)
                # u = leaky_relu(masked scores); split ACT/DVE to balance
                u = ew_sb.tile([128, GJ * NB], bf16, tag="ut")
                nc.scalar.activation(u[:, 0:NB], psum_s[:, 0:NB],
                                     Act.Lrelu, alpha=ALPHA)
                tcp = ew_sb.tile([128, NB], bf16, tag="tcp")
                nc.vector.tensor_copy(tcp[:], psum_s[:, NB:2 * NB])
                nc.vector.scalar_tensor_tensor(
                    out=u[:, NB:2 * NB], in0=tcp[:], scalar=ALPHA, in1=tcp[:],
                    op0=Alu.mult, op1=Alu.max)
                p = ew_sb.tile([128, GJ * NB], bf16, tag="pt")
                nc.scalar.activation(p[:], u[:], Act.Exp)
                for q in range(GJ):
                    jt = g * GJ + q
                    nc.tensor.matmul(psum_y[:],
                                     v_all[:, jt, b * C1:(b + 1) * C1],
                                     p[:, q * NB:(q + 1) * NB],
                                     start=(jt == 0), stop=(jt == NJT - 1))

            # ---------------- normalize + write out ----------------
            y_sb = ep_sb.tile([C1, NB], f32, tag="ysb")
            nc.vector.tensor_copy(y_sb[:], psum_y[:])
            for g4 in range(4):
                pst = ps_t.tile([128, C1], f32, tag="pst")
                nc.tensor.transpose(pst[:],
                                    y_sb[:, g4 * 128:(g4 + 1) * 128],
                                    identf[0:C1, 0:C1])
                rec = ep_sb.tile([128, 1], f32, tag="rec")
                nc.vector.reciprocal(rec[:], pst[:, C:C1])
                y_out = ep_sb.tile([128, C], f32, tag="yout")
                nc.vector.tensor_scalar(y_out[:], pst[:, 0:C], rec[:],
                                        None, Alu.mult)
                nc.sync.dma_start(out=out[b, g4 * 128:(g4 + 1) * 128, :],
                                  in_=y_out[:])


def _get_nc():
    if "nc" not in _CACHE:
        _CACHE["nc"] = _build()
    return _CACHE["nc"]


def _make_in_maps(inputs):
    x = np.asarray(inputs["x"], dtype=np.float32)
    dist_mat = np.asarray(inputs["dist_mat"], dtype=np.float32)
    proj_w = np.asarray(inputs["proj_w"], dtype=np.float32)
    proj_b = np.asarray(inputs["proj_b"], dtype=np.float32)
    a_w = np.asarray(inputs["a_w"], dtype=np.float32)

    asT = np.ascontiguousarray(a_w[:, :C].T).astype(BF16)  # [64, 4096]
    in_maps = []
    idx = np.arange(NB)
    for k in range(R):
        blk = slice(k * NB, (k + 1) * NB)
        dbT_k = np.ascontiguousarray(dist_mat[blk, :].T)   # [4096, 512]
        dbT_k[k * NB + idx, idx] = 0.25  # force diagonal -> edge (adj |= eye)
        # x packed: [c2, pair, b] with c2 = (n%2)*64 + c
        xq_k = np.ascontiguousarray(
            x[:, blk, :].reshape(B, NP, 2, C).transpose(1, 2, 3, 0)
            .reshape(NP, 2 * C, B))
        in_maps.append({
            "wb": np.ascontiguousarray(proj_w[blk].reshape(NB * C, C)),
            "xq": xq_k,
            "bbT": np.ascontiguousarray(proj_b[blk].T),
            "adT": a_w[blk, C:].astype(BF16),
            "dbT": dbT_k,
            "asT": asT,
        })
    return in_maps


def kernel(x, dist_mat, proj_w, proj_b, a_w):
    from concourse.bass_utils import run_bass_kernel_spmd

    nc = _get_nc()
    in_maps = _make_in_maps({"x": x, "dist_mat": dist_mat, "proj_w": proj_w,
                             "proj_b": proj_b, "a_w": a_w})
    last_err = None
    for _attempt in range(3):
        try:
            res = run_bass_kernel_spmd(nc, in_maps, core_ids=list(range(R)))
            outs = [res.results[k]["out"] for k in range(R)]
            return np.concatenate(outs, axis=1).astype(np.float32)
        except Exception as e:  # transient runtime/device errors: retry
            last_err = e
    raise last_err
